# revision 142
# speedup vs baseline: 1.1004x; 1.0048x over previous
"""Multi-layer GAT (2-layer graph attention network) on 8 Trainium2 NeuronCores.

Sharding: query-node rows of the NxN attention problem are sharded across the
8 cores (512 rows each); weights and the column copy of Wh are replicated.
The only collective is an AllGather of the fused layer-2 Wh_ext [4096, 67]
(fp16) between the two GAT layers.

Score math per core, transposed (j on partitions, i on free dim). Softmax
over j is invariant to any per-i scaling, so with
    gs_i = exp(0.8*src_i), ed_j = exp(dst_j), hd_j = exp(-0.8*dst_j),
    fd_j = exp(0.2*dst_j)
the exact (rescaled) numerator weight is
    T[j,i] = B[j,i] * max(gs_i, hd_j) * ed_j  =  B * max(gs_i*ed_j, fd_j)
where B is the 0/1 adjacency (column copy). The mask is applied as
t * B with B in {0, 1}, which is exact.

Per (j-tile, head) combo the hot loop runs one dual-scalar tensor_scalar
(gs max hd)*ed -- 4x DVE perf mode, or Pool for a small share -- and one
[128,1024] mask tensor_tensor multiply batched over a j-pair (DVE, with a
Pool share per TT1). PE accumulates att[f,i] over the 32 j-tiles with the
ones column of the fused Wh_ext providing the softmax denominator. Layer 1
runs as two half-loops (heads 0-1 then 2-3) so the first normalization
tails overlap the second half-loop.
"""

import numpy as np

import concourse.bacc as bacc
import concourse.mybir as mybir
from concourse.tile import TileContext
from concourse.bass_utils import run_bass_kernel_spmd
from concourse.masks import make_identity

F32 = mybir.dt.float32
BF16 = mybir.dt.bfloat16
FP16 = mybir.dt.float16
AF = mybir.ActivationFunctionType
ALU = mybir.AluOpType

N, NFEAT, NHID, NCLASS, NHEADS = 4096, 512, 64, 40, 4
NCORES = 8
NS = N // NCORES          # 512 rows (query nodes) per core
NT = N // 128             # 32 j-tiles
KX = NFEAT // 128         # 4 k-tiles over input features
C1 = NHEADS * (NHID + 3)  # 268 fused-weight cols, 67/head: src,dst,Wh[64],ones
C2 = 67                   # src2,dst2,Wo(40)+pad(24),ones at col 66
MBIG = 1.0                # multiplicative 0/1 mask value for edges

# engine-assignment knobs (tuned against TimelineSim):
# TS stage per (j, head) combo: V=DVE dual-scalar, A=Act exp + Pool max,
# P=Pool dual-scalar. TT (mask multiply) per [128,1024] pair-op: V=DVE, P=Pool.
TS1 = ['V', 'V', 'V', 'V', 'V', 'V', 'V', 'V', 'V', 'V',
       'V', 'V', 'V', 'V', 'V', 'V', 'V', 'V', 'V', 'P',
       'V', 'V', 'V', 'V', 'V', 'V', 'V', 'V', 'V', 'V',
       'V', 'V', 'V', 'V', 'V', 'V', 'V', 'V', 'P', 'V']
TT1 = ['V','V','V','P','V','V','V','V','P','V']

_compiled = None


def _build(sim_mode=False, stop_after=None):
    nc = bacc.Bacc("TRN2", num_devices=1 if sim_mode else NCORES)

    xP = nc.dram_tensor("xP", [NT + 4, 128, NFEAT], FP16, kind="ExternalInput")
    Wb = nc.dram_tensor("Wb", [NFEAT, C1], FP16, kind="ExternalInput")
    bb = nc.dram_tensor("bb", [1, C1], FP16, kind="ExternalInput")
    Wob = nc.dram_tensor("Wob", [2 * 128, C2], FP16, kind="ExternalInput")
    bob = nc.dram_tensor("bob", [1, C2], FP16, kind="ExternalInput")
    Mt = nc.dram_tensor("Mt", [128, NT, NS], FP16, kind="ExternalInput")
    y = nc.dram_tensor("y", [NS, NCLASS], F32, kind="ExternalOutput")

    with TileContext(nc) as tc:
        with (
            tc.tile_pool(name="const", bufs=1) as cp,
            tc.tile_pool(name="mask", bufs=1) as mp,
            tc.tile_pool(name="whext", bufs=1) as wp,
            tc.tile_pool(name="h1g", bufs=1) as gp,
            tc.tile_pool(name="work", bufs=2) as wk,
            tc.tile_pool(name="psA", bufs=1, space="PSUM") as psA,
            tc.tile_pool(name="dram", bufs=1, space="DRAM") as dr,
        ):
            # ---- constants / weights
            ident = cp.tile([128, 128], F32, tag="ident", name="ident")
            make_identity(nc, ident[:])
            # PE p-state warmup: ~3us of throwaway transposes while the first
            # x chunk is still in flight, so real matmuls start at full clock
            wsink = dr.tile([1, 128], F32, tag="wsink", name="wsink")
            for w in range(10):
                wtp = psA.tile([128, 128], F32, tag="bcn", name=f"warm{w}", bufs=1)
                nc.tensor.transpose(out=wtp[:], in_=ident[:], identity=ident[:])
                if w == 9:
                    wsb = cp.tile([1, 128], F32, tag="wsb", name="wsb")
                    nc.vector.tensor_copy(out=wsb[:], in_=wtp[0:1, :])
                    nc.sync.dma_start(out=wsink[:], in_=wsb[:])
            ones = cp.tile([1, 128], F32, tag="ones", name="ones")
            nc.vector.memset(ones[:], 1.0)
            ones16 = cp.tile([1, 128], FP16, tag="ones16", name="ones16")
            nc.vector.memset(ones16[:], 1.0)
            wbx = cp.tile([128, KX * C1], FP16, tag="wbx", name="wbx")
            wb_t = [wbx[:, k * C1:(k + 1) * C1] for k in range(KX)]
            bb_t = cp.tile([1, C1], FP16, tag="bb", name="bb")
            wox = cp.tile([128, 2 * C2], FP16, tag="wox", name="wox")
            wo_t = [wox[:, k * C2:(k + 1) * C2] for k in range(2)]
            bo_t = cp.tile([1, C2], FP16, tag="bo", name="bo")

            # ---- multiplicative-min mask tiles, resident through both layers
            m_all = mp.tile([128, NT * NS], FP16, tag="mall", name="mall")

            # ---- phase B: Wh_ext = x @ Wb + bb (bias as K=1 ones-row matmul).
            # 36 row-tiles: 4 own-shard (f32, for src extraction) first, then
            # all 32 node tiles (fp16 lhsT copy for attention).
            whb_all = wp.tile([128, NT * C1], FP16, tag="whball", name="whball")
            whs_all = wp.tile([128, 4 * NHEADS], F32, tag="whsall", name="whsall")
            dstc_all = wp.tile([128, NT * NHEADS], F32, tag="dstc", name="dstc")
            with tc.tile_pool(name="xpool", bufs=1) as xp:
                torder = [NT, NT + 1, NT + 2, NT + 3] + list(range(NT))
                xchunks = []
                # SP issues DMAs serially (~0.6us each): x chunks and the
                # weights lead, mask slabs pace behind the hot loop's needs
                mask_plan = {0: (0, 4), 1: (4, 8), 2: (8, 12), 3: (12, 20),
                             4: (20, 28), 5: (28, 32)}
                for c in range(6):
                    tq0, tq1 = 6 * c, min(6 * (c + 1), NT + 4)
                    xc = xp.tile([128, 6 * NFEAT], FP16, tag="xst", name="xst", bufs=3)
                    if c == 0:
                        # own tiles first in their own small transfer so the
                        # src/gs chain starts ~1.5us earlier
                        nc.sync.dma_start(
                            out=xc[:, 0:4 * NFEAT].rearrange(
                                "p (t f) -> p t f", f=NFEAT),
                            in_=xP[0:4, :, :].rearrange("t p f -> p t f"))
                        nc.sync.dma_start(
                            out=xc[:, 4 * NFEAT:6 * NFEAT].rearrange(
                                "p (t f) -> p t f", f=NFEAT),
                            in_=xP[4:6, :, :].rearrange("t p f -> p t f"))
                    else:
                        nc.sync.dma_start(
                            out=xc[:, 0:(tq1 - tq0) * NFEAT].rearrange(
                                "p (t f) -> p t f", f=NFEAT),
                            in_=xP[tq0:tq1, :, :].rearrange("t p f -> p t f"))
                    xchunks.append(xc)
                    if c == 0:
                        nc.sync.dma_start(
                            out=wbx[:].rearrange("p (k c) -> p k c", c=C1),
                            in_=Wb[:, :].rearrange("(k p) c -> p k c", p=128))
                        nc.sync.dma_start(out=bb_t[:], in_=bb[:])
                    j0, j1 = mask_plan[c]
                    for jj in range(j0, j1, 4):
                        jh = min(jj + 4, j1)
                        nc.sync.dma_start(out=m_all[:, jj * NS:jh * NS],
                                          in_=Mt[:, jj:jh, :])
                    if c == 5:
                        nc.sync.dma_start(
                            out=wox[:].rearrange("p (k c) -> p k c", c=C2),
                            in_=Wob[:, :].rearrange("(k p) c -> p k c", p=128))
                        nc.sync.dma_start(out=bo_t[:], in_=bob[:])
                # 2-deep PSUM staging (copy of tile k overlaps matmuls of
                # k+1); att banks stay free so the hot loop can start while
                # phase B is still streaming
                cp_rot = 0
                for tq in range(36):
                    t = torder[tq]
                    ps = psA.tile([128, C1], F32, tag="mmps", name=f"st{tq}",
                                  bufs=2)
                    xk = xchunks[tq // 6][:, (tq % 6) * NFEAT:(tq % 6 + 1) * NFEAT]
                    for k in range(KX):
                        nc.tensor.matmul(out=ps[:], lhsT=xk[:, k * 128:(k + 1) * 128],
                                         rhs=wb_t[k][:],
                                         start=(k == 0), stop=False)
                    nc.tensor.matmul(out=ps[:], lhsT=ones16[:], rhs=bb_t[:],
                                     start=False, stop=True)
                    if t >= NT:
                        # own-shard tiles only feed the src broadcast: copy
                        # just the 4 src columns (full matmul stays as PE
                        # p-state ramp fuel)
                        nc.vector.tensor_copy(
                            out=whs_all[:, (t - NT) * NHEADS:(t - NT + 1) * NHEADS],
                            in_=ps[:, 0:1 + 67 * (NHEADS - 1):67])
                    else:
                        dst1 = whb_all[:, t * C1:(t + 1) * C1]
                        nc.scalar.copy(out=dst1, in_=ps[:])
                        cp_rot += 1
                        if t % 8 == 7:
                            # dst columns (col 1 of each 67-block): C1 = 4*67,
                            # so stride 67 runs continuously across tiles
                            q = t - 7
                            nc.gpsimd.tensor_copy(
                                out=dstc_all[:, q * NHEADS:(t + 1) * NHEADS],
                                in_=whb_all[:, q * C1 + 1:q * C1 + 2 + 31 * 67:67])

            # ---- phase C: src broadcasts (PE transpose) + exp precomputes
            gs_b = [cp.tile([128, NS], FP16, tag=f"gsb{h}", name=f"gsb{h}")
                    for h in range(NHEADS)]
            for h in range(NHEADS):
                bc = psA.tile([128, 512], F32, tag="bc", name="bc", bufs=1)
                for t in range(4):
                    nc.tensor.transpose(
                        out=bc[:, t * 128:(t + 1) * 128],
                        in_=whs_all[:, t * NHEADS + h:t * NHEADS + h + 1].to_broadcast(
                            [128, 128]),
                        identity=ident[:])
                nc.scalar.activation(gs_b[h][:], bc[:], AF.Exp, scale=0.8)
            # chunked so hot-loop tile j only waits on its own octet of tiles
            ed_all = cp.tile([128, NT * NHEADS], F32, tag="ed", name="ed")
            hd_all = cp.tile([128, NT * NHEADS], F32, tag="hd", name="hd")
            fd_all = cp.tile([128, NT * NHEADS], F32, tag="fd", name="fd")
            for q in range(4):
                sl_ = slice(q * 8 * NHEADS, (q + 1) * 8 * NHEADS)
                nc.scalar.activation(ed_all[:, sl_], dstc_all[:, sl_], AF.Exp)
                nc.scalar.activation(hd_all[:, sl_], dstc_all[:, sl_], AF.Exp, scale=-0.8)
                nc.scalar.activation(fd_all[:, sl_], dstc_all[:, sl_], AF.Exp, scale=0.2)

            h1t = [cp.tile([128, NS], FP16, tag=f"h1t{i}", name=f"h1t{i}") for i in range(2)]

            # ---- phase D: layer-1 attention in two half-loops (heads 0-1,
            # then heads 2-3) so the first tails overlap the second half-loop
            att = [psA.tile([NHID + 1, NS], F32, tag=f"att{h}", name=f"att{h}", bufs=1)
                   for h in range(NHEADS)]

            def l1_tail(h):
                bcn = psA.tile([NHID, NS], F32, tag="bcn", name=f"bcn{h}", bufs=1)
                for q in range(2):
                    sl_ = slice(q * 256, (q + 1) * 256)
                    rec = wk.tile([1, 256], F32, tag="rec", name=f"rec{h}{q}", bufs=4)
                    nc.vector.reciprocal(out=rec[:], in_=att[h][NHID:NHID + 1, sl_])
                    nc.tensor.matmul(out=bcn[:, sl_], lhsT=ones[:, 0:NHID],
                                     rhs=rec[:], start=True, stop=True)
                    nsb = wk.tile([NHID, 256], F32, tag="nsb1", name=f"nsb1_{h}{q}",
                                  bufs=4)
                    nc.scalar.copy(out=nsb[:], in_=att[h][0:NHID, sl_])
                    pre = wk.tile([NHID, 256], F32, tag="pre", name=f"pre{h}{q}",
                                  bufs=4)
                    nc.vector.tensor_tensor(out=pre[:], in0=nsb[:], in1=bcn[:, sl_],
                                            op=ALU.mult)
                    # elu(x) = min(exp(x),1) - 1 + relu(x)
                    ex = wk.tile([NHID, 256], F32, tag="ex", name=f"ex{h}{q}", bufs=4)
                    nc.scalar.activation(ex[:], pre[:], AF.Exp)
                    rm1 = wk.tile([NHID, 256], F32, tag="rm1", name=f"rm1{h}{q}",
                                  bufs=4)
                    nc.gpsimd.tensor_scalar(out=rm1[:], in0=pre[:], scalar1=0.0,
                                            scalar2=-1.0, op0=ALU.max, op1=ALU.add)
                    nc.vector.scalar_tensor_tensor(
                        out=h1t[h // 2][64 * (h % 2):64 * (h % 2) + 64, sl_],
                        in0=ex[:], scalar=1.0, in1=rm1[:], op0=ALU.min, op1=ALU.add)

            # per-combo engine choice; each (j-pair, head) does:
            #   t[j0],t[j1] via TS (DVE dual-scalar / Act exp+bias [+Pool max] /
            #   Pool dual-scalar), then one [128,1024] mask TT (DVE or Pool).
            def l1_ts(eng, dst_ap, h, c, scratch):
                e = nc.vector if eng == 'V' else nc.gpsimd
                e.tensor_scalar(
                    out=dst_ap, in0=gs_b[h][:],
                    scalar1=hd_all[:, c:c + 1], scalar2=ed_all[:, c:c + 1],
                    op0=ALU.max, op1=ALU.mult)

            for half in range(2):
                hs = (2 * half, 2 * half + 1)
                for pp in range(NT // 2):
                    j0 = 2 * pp
                    if half == 1 and pp == 6:
                        # half-0 tails go here so their att-stop waits don't
                        # head-of-line-block half-1's queue entry
                        l1_tail(0)
                        l1_tail(1)
                    for u, h in enumerate(hs):
                        t_pr = wk.tile([128, 2 * NS], FP16, tag="t_all",
                                       name="t_all", bufs=10)
                        T_pr = wk.tile([128, 2 * NS], FP16, tag="T_all",
                                       name="T_all", bufs=10)
                        for v in range(2):
                            j = j0 + v
                            c = j * NHEADS + h
                            l1_ts(TS1[(j * NHEADS + h) % len(TS1)],
                                  t_pr[:, v * NS:(v + 1) * NS], h, c, None)
                        tte = nc.vector if TT1[(pp * 4 + h) % len(TT1)] == 'V' \
                            else nc.gpsimd
                        tte.tensor_tensor(
                            out=T_pr[:], in0=t_pr[:],
                            in1=m_all[:, j0 * NS:(j0 + 2) * NS], op=ALU.mult)
                        for v in range(2):
                            j = j0 + v
                            nc.tensor.matmul(
                                out=att[h][:],
                                lhsT=whb_all[:, j * C1 + 67 * h + 2:
                                             j * C1 + 67 * h + 67],
                                rhs=T_pr[:, v * NS:(v + 1) * NS],
                                start=(j == 0), stop=(j == NT - 1))
                if half == 1:
                    l1_tail(2)
                    l1_tail(3)

            # ---- phase F: Wh2 for own shard only; AllGather the [512, C2]
            # fused result (fp16) instead of gathering h1 itself.
            whs2_all = wp.tile([128, 4], F32, tag="whs2all", name="whs2all")
            agin2 = dr.tile([4 * 128, C2], FP16, tag="agin2", name="agin2")
            agout2 = dr.tile([NCORES * 4 * 128, C2], FP16, tag="agout2", name="agout2")
            whsb = wk.tile([128, 4 * C2], FP16, tag="whsb", name="whsb", bufs=1)
            for t in range(4):
                ps = psA.tile([128, C1], F32, tag="mmps", name=f"mmps2_{t}", bufs=2)
                half = ps[:, 0:C2]
                for k in range(2):
                    nc.tensor.matmul(out=half, lhsT=h1t[k][:, t * 128:(t + 1) * 128],
                                     rhs=wo_t[k][:], start=(k == 0), stop=False)
                nc.tensor.matmul(out=half, lhsT=ones16[:], rhs=bo_t[:],
                                 start=False, stop=True)
                nc.vector.tensor_copy(
                    out=whs2_all[:, t:t + 1], in_=half[:, 0:1])
                nc.vector.tensor_copy(out=whsb[:, t * C2:(t + 1) * C2], in_=half)
            nc.sync.dma_start(
                out=agin2[:].rearrange("(t p) c -> p t c", p=128),
                in_=whsb[:].rearrange("p (t c) -> p t c", c=C2))
            wh2all = gp.tile([128, NT * C2], FP16, tag="wh2all", name="wh2all")
            if sim_mode:
                # split the broadcast so the first wh2all half and its dst
                # precomputes overlap the second half's transfer
                for hh in range(4):
                    nc.sync.dma_start(
                        out=agout2[hh * 1024:(hh + 1) * 1024, :].rearrange(
                            "(r q) c -> r q c", r=NCORES // 4),
                        in_=agin2[:].rearrange("(x q) c -> x q c", x=1).to_broadcast(
                            [NCORES // 4, 4 * 128, C2]))
            else:
                nc.gpsimd.collective_compute(
                    "AllGather", ALU.bypass,
                    replica_groups=[list(range(NCORES))],
                    ins=[agin2[:].opt()],
                    outs=[agout2[:].opt()])
            dst2c = gp.tile([128, NT], F32, tag="dst2c", name="dst2c")
            ed2 = gp.tile([128, NT], F32, tag="ed2", name="ed2")
            hd2 = gp.tile([128, NT], F32, tag="hd2", name="hd2")
            fd2 = gp.tile([128, NT], F32, tag="fd2", name="fd2")
            for hh in range(4):
                qsl = slice(hh * 8, (hh + 1) * 8)
                nc.sync.dma_start(
                    out=wh2all[:, hh * 8 * C2:(hh + 1) * 8 * C2].rearrange(
                        "p (q c) -> p q c", c=C2),
                    in_=agout2[hh * 1024:(hh + 1) * 1024, :].rearrange(
                        "(q p) c -> p q c", p=128))
                base = hh * 8 * C2
                nc.vector.tensor_copy(
                    out=dst2c[:, qsl],
                    in_=wh2all[:, base + 1:base + 2 + 7 * C2:C2])
                nc.scalar.activation(ed2[:, qsl], dst2c[:, qsl], AF.Exp)
                nc.scalar.activation(hd2[:, qsl], dst2c[:, qsl], AF.Exp, scale=-0.8)
                nc.scalar.activation(fd2[:, qsl], dst2c[:, qsl], AF.Exp, scale=0.2)

            # ---- phase G: src2 broadcast + exp
            gs2b = cp.tile([128, NS], FP16, tag="gs2b", name="gs2b")
            bc = psA.tile([128, 512], F32, tag="bc", name="bc2", bufs=1)
            for t in range(4):
                nc.tensor.transpose(
                    out=bc[:, t * 128:(t + 1) * 128],
                    in_=whs2_all[:, t:t + 1].to_broadcast([128, 128]),
                    identity=ident[:])
            nc.scalar.activation(gs2b[:], bc[:], AF.Exp, scale=0.8)
            # keep PE warm across the collective gap so layer-2 matmuls start
            # at full clock
            for w in range(3):
                wtp = psA.tile([128, 128], F32, tag="bc", name=f"warm2_{w}", bufs=1)
                nc.tensor.transpose(out=wtp[:], in_=ident[:], identity=ident[:])
                if w == 2:
                    wsb2 = wk.tile([1, 128], F32, tag="wsb2", name="wsb2", bufs=1)
                    nc.vector.tensor_copy(out=wsb2[:], in_=wtp[0:1, :])
                    nc.sync.dma_start(out=wsink[:], in_=wsb2[:])

            # ---- phase H: layer-2 attention (single head)
            att2 = psA.tile([NHID + 1, NS], F32, tag="att0", name="att2", bufs=1)
            for pp in range(NT // 2):
                j0 = 2 * pp
                t2 = wk.tile([128, 2 * NS], FP16, tag="t2", name="t2", bufs=6)
                T2 = wk.tile([128, 2 * NS], FP16, tag="T2", name="T2", bufs=6)
                for u in range(2):
                    j = j0 + u
                    e = nc.gpsimd if j % 4 >= 2 else nc.vector
                    e.tensor_scalar(
                        out=t2[:, u * NS:(u + 1) * NS], in0=gs2b[:],
                        scalar1=hd2[:, j:j + 1], scalar2=ed2[:, j:j + 1],
                        op0=ALU.max, op1=ALU.mult)
                tte = nc.vector
                tte.tensor_tensor(
                    out=T2[:], in0=t2[:],
                    in1=m_all[:, j0 * NS:(j0 + 2) * NS], op=ALU.mult)
                for u in range(2):
                    j = j0 + u
                    nc.tensor.matmul(
                        out=att2[0:65, :], lhsT=wh2all[:, j * C2 + 2:(j + 1) * C2],
                        rhs=T2[:, u * NS:(u + 1) * NS],
                        start=(j == 0), stop=(j == NT - 1))
            rec = wk.tile([1, NS], F32, tag="rec2f", name="rec2", bufs=1)
            nc.vector.reciprocal(out=rec[:], in_=att2[64:65, :])
            nsb = wk.tile([NCLASS, NS], F32, tag="nsb", name="nsb2", bufs=2)
            nc.scalar.copy(out=nsb[:], in_=att2[0:NCLASS, :])
            # elu outputs are >= -1 and |out| is small, so log_softmax needs no
            # max-subtraction: y = z - ln(sum(exp(z))). All Exp ops first, the
            # four Ln ops last (one activation-table switch instead of eight).
            zs = []
            ssum_all = wk.tile([128, 4], F32, tag="ssumall", name="ssumall", bufs=1)
            yt_all = wk.tile([128, 4 * NCLASS], F32, tag="ytall", name="ytall", bufs=1)
            for c in range(4):
                tp = psA.tile([128, NCLASS], F32, tag="bc", name="tr", bufs=1)
                nc.tensor.transpose(out=tp[:], in_=nsb[:, c * 128:(c + 1) * 128],
                                    identity=ident[0:NCLASS, 0:NCLASS])
                tpr = psA.tile([128, 1], F32, tag="bcn", name="trr", bufs=1)
                nc.tensor.transpose(out=tpr[:], in_=rec[:, c * 128:(c + 1) * 128],
                                    identity=ident[0:1, 0:1])
                rcol = wk.tile([128, 1], F32, tag="rcol", name="rcol", bufs=2)
                nc.vector.tensor_copy(out=rcol[:], in_=tpr[:])
                # o2t = numerator^T * (1/denom), per-partition scalar
                o2t = wk.tile([128, NCLASS], F32, tag="z", name="o2t", bufs=2)
                nc.vector.tensor_scalar(out=o2t[:], in0=tp[:], scalar1=rcol[:, 0:1],
                                        scalar2=None, op0=ALU.mult)
                # elu = min(exp(x),1) - 1 + relu(x)
                exv = wk.tile([128, NCLASS], F32, tag="ez", name="exv", bufs=2)
                nc.scalar.activation(exv[:], o2t[:], AF.Exp)
                rm1 = wk.tile([128, NCLASS], F32, tag="rm1c", name="rm1c", bufs=2)
                nc.gpsimd.tensor_scalar(out=rm1[:], in0=o2t[:], scalar1=0.0, scalar2=-1.0,
                                        op0=ALU.max, op1=ALU.add)
                z = wk.tile([128, NCLASS], F32, tag="zc", name="zc", bufs=4)
                nc.vector.scalar_tensor_tensor(out=z[:], in0=exv[:], scalar=1.0, in1=rm1[:],
                                               op0=ALU.min, op1=ALU.add)
                ez = wk.tile([128, NCLASS], F32, tag="ez2", name="ez", bufs=2)
                nc.scalar.activation(ez[:], z[:], AF.Exp, accum_out=ssum_all[:, c:c + 1])
                zs.append(z)
            # one Ln over all four chunks' sums: a single activation-table
            # switch, guaranteed to run after every Exp
            ls = wk.tile([128, 4], F32, tag="ls", name="ls", bufs=1)
            nc.scalar.activation(ls[:], ssum_all[:], AF.Ln)
            for c in range(4):
                nc.vector.tensor_scalar(out=yt_all[:, c * NCLASS:(c + 1) * NCLASS],
                                        in0=zs[c][:], scalar1=ls[:, c:c + 1],
                                        scalar2=None, op0=ALU.subtract)
            nc.sync.dma_start(
                out=y[:, :].rearrange("(c p) n -> p c n", p=128),
                in_=yt_all[:].rearrange("p (c n) -> p c n", n=NCLASS))

    nc.compile()
    return nc


def _prep_inputs(x, edge_index, W1, b1, a1, ab1, Wo, bo, ao, abo):
    x = np.asarray(x, np.float32)
    W1 = np.asarray(W1, np.float32)
    b1 = np.asarray(b1, np.float32)
    a1 = np.asarray(a1, np.float32)
    ab1 = np.asarray(ab1, np.float32)
    Wo = np.asarray(Wo, np.float32)
    bo = np.asarray(bo, np.float32)
    ao = np.asarray(ao, np.float32)
    abo = np.asarray(abo, np.float32)

    # packed lhsT tiles: xP[t, p, k*128+c] = x[node = t*128+c, feat = k*128+p]
    x16 = x.astype(np.float16)
    xall = x16.reshape(NT, 128, KX, 128).transpose(0, 3, 2, 1).reshape(NT, 128, NFEAT)

    Wb = np.zeros((NFEAT, C1), np.float32)
    bbrow = np.zeros((C1,), np.float32)
    for h in range(NHEADS):
        c = 67 * h
        Wb[:, c + 0] = W1[h] @ a1[h, :NHID]
        Wb[:, c + 1] = W1[h] @ a1[h, NHID:]
        Wb[:, c + 2:c + 2 + NHID] = W1[h]
        bbrow[c + 0] = b1[h] @ a1[h, :NHID]
        bbrow[c + 1] = b1[h] @ a1[h, NHID:] + ab1[h]
        bbrow[c + 2:c + 2 + NHID] = b1[h]
        bbrow[c + 66] = 1.0
    bb = bbrow[None, :].copy()

    Wob = np.zeros((2 * 128, C2), np.float32)
    Wob[:, 0] = Wo @ ao[:NCLASS]
    Wob[:, 1] = Wo @ ao[NCLASS:]
    Wob[:, 2:2 + NCLASS] = Wo
    borow = np.zeros((C2,), np.float32)
    borow[0] = bo @ ao[:NCLASS]
    borow[1] = bo @ ao[NCLASS:] + abo
    borow[2:2 + NCLASS] = bo
    borow[66] = 1.0
    bob = borow[None, :].copy()

    # multiplicative-min mask, transposed: M[j, i] = 65504 if adj[i, j] else 0
    ei = np.asarray(edge_index).astype(np.int64)
    Mbig = np.zeros((N, N), np.float16)
    Mbig[ei[1], ei[0]] = MBIG

    in_maps = []
    for c in range(NCORES):
        sl = slice(c * NS, (c + 1) * NS)
        xs = x16[sl].reshape(4, 128, KX, 128).transpose(0, 3, 2, 1).reshape(4, 128, NFEAT)
        in_maps.append({
            "xP": np.ascontiguousarray(np.concatenate([xs, xall], axis=0)),
            "Wb": Wb.astype(np.float16), "bb": bb.astype(np.float16),
            "Wob": Wob.astype(np.float16), "bob": bob.astype(np.float16),
            "Mt": np.ascontiguousarray(
                Mbig[:, sl].reshape(NT, 128, NS).transpose(1, 0, 2)),
        })
    return in_maps


def kernel(x, edge_index, W1, b1, a1, ab1, Wo, bo, ao, abo, _trace=False):
    global _compiled
    if _compiled is None:
        _compiled = _build()
    in_maps = _prep_inputs(x, edge_index, W1, b1, a1, ab1, Wo, bo, ao, abo)
    res = run_bass_kernel_spmd(_compiled, in_maps, core_ids=list(range(NCORES)),
                               trace=_trace)
    kernel.last_result = res
    return np.concatenate([res.results[c]["y"] for c in range(NCORES)], axis=0)
